# revision 3
# baseline (speedup 1.0000x reference)
"""Trainium2 Bass kernel for nn_Attention (pooling attention).

Math (per batch b):
    u[b]     = W_score @ h_t[b]            (score = (hidden @ W_score) . h_t
                                            collapses to hidden . (W_score @ h_t))
    score[t] = hidden[b,t,:] . u[b]        (DVE fp16 mul + pairwise-add tree)
    p[t]     = exp(score[t] - 50)          (ScalarE, fused per-partition accum -> q)
    s        = sum_t p[t]                  (PE ones-matmul over q -> s on all parts)
    w[t]     = p[t] / s                    (ACT copy with scale=1/s, fp16-safe)
    ctx      = sum_t w[t] * hidden[b,t,:]  (PE fp16: w column as 1-col stationary)
    out[b]   = tanh([ctx, h_t[b]] @ W_att)

Sharding: data-parallel over batch, 16 batches per core on 8 cores; weights
replicated.  hidden_states is read from HBM exactly once (fp32), cast to fp16
during the DMA (SWDGE cast), and never transposed.

Pipeline design (vs the earlier version):
  - The y16 load flood starts immediately; the small setup DMAs (ident, ht,
    wst, watt) ride the sync HWDGE ring concurrently (no dependent DMAs on
    that ring, so no FIFO blocking).
  - u broadcast to all partitions via K=1 PE matmuls (ones_row (x) u[b]) +
    ACT copies -- no SBUF->SBUF DMAs stealing SDMA bandwidth from the flood.
  - softmax sum via PE ones-matmul (s broadcast to all 128 partitions in one
    matmul); no gpsimd partition_all_reduce (avoids DVE shared-port jitter).
  - normalize w = p * (1/s) on ACT (Copy with per-partition scale AP).
  - reciprocal stays on DVE but is emitted AFTER the next batch's big mul so
    the DVE FIFO never head-of-line blocks on the exp->sum chain.
  - last batch's load is split into 4 chunks so its score pipeline overlaps
    the tail of the flood.
"""

import sys

import numpy as np

_TRN_REPO = "/opt/trn_rl_repo"
if _TRN_REPO not in sys.path:
    sys.path.insert(0, _TRN_REPO)

import concourse.bass as bass
import concourse.bacc as bacc
import concourse.tile as tile
from concourse import mybir
from concourse.bass_utils import run_bass_kernel_spmd

N_CORES = 8
B, T, H = 128, 2048, 256
NB = B // N_CORES  # batches per core
P = 128  # SBUF partitions
TT = T // P  # t-tiles per batch
OUT_D = 128
EXP_SHIFT = -50.0  # keeps exp() in fp32 range; cancels in the softmax ratio

NCH = 4  # last batch is loaded/scored in NCH chunks to shorten the tail
CTT = TT // NCH

F32 = mybir.dt.float32
F16 = mybir.dt.float16


def _build_kernel(nc: bass.Bass, tc: "tile.TileContext", hidden, wst, watt, ident, out):
    add = mybir.AluOpType.add

    from contextlib import ExitStack

    with ExitStack() as ctx:
        const = ctx.enter_context(tc.tile_pool(name="const", bufs=1))
        ybufs = ctx.enter_context(tc.tile_pool(name="ybufs", bufs=6))
        sc = ctx.enter_context(tc.tile_pool(name="sc", bufs=3))
        psum_t = ctx.enter_context(tc.tile_pool(name="psum_t", bufs=2, space="PSUM"))
        psum_u = ctx.enter_context(tc.tile_pool(name="psum_u", bufs=2, space="PSUM"))
        psum_p = ctx.enter_context(tc.tile_pool(name="psum_p", bufs=1, space="PSUM"))

        # ---- constants (no DMA needed) -------------------------------------
        ones_row16 = const.tile([1, P], F16, tag="ones_row16")
        nc.vector.memset(ones_row16, 1.0)
        ones128 = const.tile([P, P], F32, tag="ones128")
        nc.vector.memset(ones128, 1.0)
        ones_col1 = const.tile([1, 1], F32, tag="ones_col1")
        nc.vector.memset(ones_col1, 1.0)
        shift_col = const.tile([P, 1], F32, tag="shift_col")
        nc.vector.memset(shift_col, EXP_SHIFT)

        # ---- setup DMAs: all independent, all on the sync HWDGE ring -------
        ident_sb = const.tile([16, 16], F32, tag="ident")
        nc.sync.dma_start(out=ident_sb, in_=ident[:, :])
        ht_sb = const.tile([NB, H], F32, tag="ht")
        nc.sync.dma_start(out=ht_sb, in_=hidden[:, T - 1, :])
        wst_sb = const.tile([P, 2, H], F32, tag="wst")  # W_score^T as [k, kk, h]
        nc.sync.dma_start(out=wst_sb, in_=wst.rearrange("(kk p) h -> p kk h", p=P))
        watt_sb = const.tile([P, 4, OUT_D], F32, tag="watt")  # W_att as [d, dd, j]
        nc.sync.dma_start(out=watt_sb, in_=watt.rearrange("(dd p) j -> p dd j", p=P))

        # ---- y16 load flood (SWDGE cast fp32->fp16), starts immediately ----
        ylist = {}
        for k in range(NB - 1):
            y = ybufs.tile([P, TT, H], F16, tag="y16", name=f"y16_{k}")
            nc.gpsimd.dma_start(
                out=y, in_=hidden[k].rearrange("(p i) h -> p i h", i=TT)
            )
            ylist[k] = y
        ychunks = []
        hlast = hidden[NB - 1].rearrange("(p i) h -> p i h", i=TT)
        for c in range(NCH):
            yc = ybufs.tile([P, CTT, H], F16, tag="y16c", name=f"y16c_{c}")
            nc.gpsimd.dma_start(out=yc, in_=hlast[:, c * CTT : (c + 1) * CTT, :])
            ychunks.append(yc)

        # ---- h_t^T and u = h_t @ W_score^T (prologue, overlaps the flood) --
        htT_sb = const.tile([P, 2, NB], F32, tag="htT")  # h_t^T halves [k, half, b]
        for half in range(2):
            ps_tr = psum_t.tile([P, NB], F32, tag="ptmp", name=f"ps_tr{half}")
            nc.tensor.matmul(
                ps_tr,
                lhsT=ht_sb[:, half * P : (half + 1) * P],
                rhs=ident_sb,
                start=True,
                stop=True,
            )
            nc.scalar.copy(out=htT_sb[:, half, :], in_=ps_tr)

        ps_u = psum_t.tile([NB, H], F32, tag="ptmp")
        for half in range(2):
            nc.tensor.matmul(
                ps_u,
                lhsT=htT_sb[:, half, :],
                rhs=wst_sb[:, half, :],
                start=(half == 0),
                stop=(half == 1),
            )
        u16_sb = const.tile([NB, H], F16, tag="u16")
        nc.scalar.copy(out=u16_sb, in_=ps_u)

        # gather all u rows onto partition 0 (matmul operands must start at
        # partition 0/32/64) -- one tiny SBUF->SBUF DMA, 8KB
        u_row = const.tile([1, NB, H], F16, tag="u_row")
        nc.sync.dma_start(out=u_row, in_=u16_sb)

        # per-batch broadcast of u[b] to all 128 partitions via K=1 matmuls
        # (ones_row16^T @ u_row[b]) -> PSUM -> ACT copy to SBUF fp16.
        ubc_all = const.tile([P, NB, H], F16, tag="ubc_all")
        for b in range(NB):
            ps_ubc = psum_u.tile([P, H], F32, tag="pubc", name=f"pubc{b}")
            nc.tensor.matmul(
                ps_ubc,
                lhsT=ones_row16,
                rhs=u_row[0:1, b, :],
                start=True,
                stop=True,
            )
            nc.scalar.copy(out=ubc_all[:, b, :], in_=ps_ubc)

        # ---- persistent PSUM accumulators for ctx^T ------------------------
        ctxT_ps = [
            psum_p.tile([P, NB], F32, tag=f"ctxT{j}", name=f"ctxT{j}")
            for j in range(2)
        ]

        # ---- per-batch score pipeline --------------------------------------
        # t = p*TT + i block mapping gives 16KB-contiguous DMA runs per
        # partition (softmax/context are t-permutation-invariant).
        state = {}  # batch -> dict of tiles needed by the deferred stages

        def ubc_rep(b, rep):
            ubc = ubc_all[:, b, :]
            return bass.AP(
                tensor=ubc.tensor,
                offset=ubc.offset,
                ap=[list(ubc.ap[0]), [0, rep], list(ubc.ap[1])],
            )

        def emit_score_tail(b, z, score, nt):
            # z1/z2 pairwise tree + fp32 reduce, then exp with accumulation
            z1 = sc.tile([P, nt, 128], F16, tag="z1" if nt == TT else "z1c")
            nc.vector.tensor_add(z1, z[:, :, 0:128], z[:, :, 128:256])
            z2 = sc.tile([P, nt, 64], F16, tag="z2" if nt == TT else "z2c")
            nc.vector.tensor_add(z2, z1[:, :, 0:64], z1[:, :, 64:128])
            nc.vector.tensor_reduce(
                out=score, in_=z2, axis=mybir.AxisListType.X, op=add
            )

        def emit_exp_s(b, score):
            p_t = sc.tile([P, TT], F32, tag="p")
            q = sc.tile([P, 1], F32, tag="q")
            nc.scalar.activation(
                out=p_t,
                in_=score,
                func=mybir.ActivationFunctionType.Exp,
                bias=shift_col,
                scale=1.0,
                accum_out=q,
            )
            s_ps = psum_u.tile([P, 1], F32, tag="ps_s")
            nc.tensor.matmul(s_ps, lhsT=ones128, rhs=q, start=True, stop=True)
            state[b] = {"p": p_t, "s_ps": s_ps}

        def emit_recip(b):
            rs = sc.tile([P, 1], F32, tag="rs")
            nc.vector.reciprocal(out=rs, in_=state[b]["s_ps"])
            state[b]["rs"] = rs

        def emit_w16(b):
            w16 = sc.tile([P, TT], F16, tag="w16")
            nc.scalar.mul(out=w16, in_=state[b]["p"], mul=state[b]["rs"])
            state[b]["w16"] = w16

        def emit_ctx(b):
            w16 = state[b]["w16"]
            ctx_ps = psum_t.tile([1, H], F32, tag="ptmp", name=f"ctx{b}")
            if b < NB - 1:
                y16 = ylist.pop(b)
                for i in range(TT):
                    nc.tensor.matmul(
                        ctx_ps,
                        lhsT=w16[:, i : i + 1],
                        rhs=y16[:, i, :],
                        start=(i == 0),
                        stop=(i == TT - 1),
                    )
            else:
                for i in range(TT):
                    nc.tensor.matmul(
                        ctx_ps,
                        lhsT=w16[:, i : i + 1],
                        rhs=ychunks[i // CTT][:, i % CTT, :],
                        start=(i == 0),
                        stop=(i == TT - 1),
                    )
            state[b]["ctx_ps"] = ctx_ps

        def emit_ctx_row(b):
            ctx_row = sc.tile([1, H], F32, tag="ctx_row")
            nc.scalar.copy(out=ctx_row, in_=state[b]["ctx_ps"])
            state[b]["ctx_row"] = ctx_row

        def emit_scatter(b):
            ctx_row = state[b]["ctx_row"]
            for j in range(2):
                nc.tensor.matmul(
                    ctxT_ps[j][:, b : b + 1],
                    lhsT=ctx_row[:, j * P : (j + 1) * P],
                    rhs=ones_col1,
                    start=True,
                    stop=True,
                )
            del state[b]

        for b in range(NB):
            if b < NB - 1:
                # one big mul covers the exp->sum latency of batch b-1, so
                # recip(b-1) never stalls the DVE FIFO
                z = sc.tile([P, TT, H], F16, tag="z")
                nc.vector.tensor_mul(z, ylist[b], ubc_rep(b, TT))
                if b >= 1:
                    emit_recip(b - 1)
                    emit_w16(b - 1)
                    emit_ctx(b - 1)
                score = sc.tile([P, TT], F32, tag="score")
                emit_score_tail(b, z, score, TT)
            else:
                score = sc.tile([P, TT], F32, tag="score")
                for c in range(NCH):
                    zc = sc.tile([P, CTT, H], F16, tag="zc")
                    nc.vector.tensor_mul(zc, ychunks[c], ubc_rep(b, CTT))
                    if c == 0:
                        emit_recip(b - 1)
                        emit_w16(b - 1)
                        emit_ctx(b - 1)
                    emit_score_tail(b, zc, score[:, c * CTT : (c + 1) * CTT], CTT)
            emit_exp_s(b, score)
            if b >= 1:
                emit_ctx_row(b - 1)
                emit_scatter(b - 1)

        bl = NB - 1
        emit_recip(bl)
        emit_w16(bl)
        emit_ctx(bl)
        emit_ctx_row(bl)
        emit_scatter(bl)

        # ---- finalize: concat with h_t, @W_att, tanh -----------------------
        preT = sc.tile([P, 2, NB], F32, tag="preT")
        for j in range(2):
            nc.scalar.copy(out=preT[:, j, :], in_=ctxT_ps[j])

        out_ps = psum_t.tile([NB, OUT_D], F32, tag="ptmp")
        for dd in range(4):
            lhsT = preT[:, dd, :] if dd < 2 else htT_sb[:, dd - 2, :]
            nc.tensor.matmul(
                out_ps,
                lhsT=lhsT,
                rhs=watt_sb[:, dd, :],
                start=(dd == 0),
                stop=(dd == 3),
            )
        out_sb = sc.tile([NB, OUT_D], F32, tag="out_sb")
        nc.scalar.activation(
            out=out_sb, in_=out_ps, func=mybir.ActivationFunctionType.Tanh
        )
        nc.sync.dma_start(out=out[:, :], in_=out_sb)


_NC_CACHE = {}


def _get_nc():
    if "nc" not in _NC_CACHE:
        nc = bacc.Bacc("TRN2", target_bir_lowering=False, debug=False)
        hidden = nc.declare_dram_parameter("hidden", [NB, T, H], F32, isOutput=False)
        wst = nc.declare_dram_parameter("w_score_t", [H, H], F32, isOutput=False)
        watt = nc.declare_dram_parameter("w_att", [2 * H, OUT_D], F32, isOutput=False)
        ident = nc.declare_dram_parameter("ident16", [16, 16], F32, isOutput=False)
        out = nc.declare_dram_parameter("out", [NB, OUT_D], F32, isOutput=True)
        with tile.TileContext(nc) as tc:
            _build_kernel(nc, tc, hidden, wst, watt, ident, out)
        nc.compile()
        _NC_CACHE["nc"] = nc
    return _NC_CACHE["nc"]


def _run(hidden_states, W_score, W_att, trace=False, trace_kwargs=None):
    hidden_states = np.ascontiguousarray(np.asarray(hidden_states, dtype=np.float32))
    W_score = np.asarray(W_score, dtype=np.float32)
    W_att = np.ascontiguousarray(np.asarray(W_att, dtype=np.float32))
    wst = np.ascontiguousarray(W_score.T)
    ident = np.eye(16, dtype=np.float32)

    nc = _get_nc()
    in_maps = []
    for c in range(N_CORES):
        in_maps.append(
            {
                "hidden": hidden_states[c * NB : (c + 1) * NB],
                "w_score_t": wst,
                "w_att": W_att,
                "ident16": ident,
            }
        )
    kwargs = {}
    if trace:
        kwargs["trace"] = True
        if trace_kwargs:
            kwargs.update(trace_kwargs)
    res = run_bass_kernel_spmd(nc, in_maps, list(range(N_CORES)), **kwargs)
    out = np.concatenate([res.results[c]["out"] for c in range(N_CORES)], axis=0)
    return out, res


def kernel(hidden_states, W_score, W_att):
    out, _ = _run(hidden_states, W_score, W_att, trace=False)
    return out


# revision 6
# speedup vs baseline: 1.2346x; 1.2346x over previous
"""Trainium2 Bass kernel for nn_Attention (pooling attention).

Math (per batch b):
    u[b]     = W_score @ h_t[b]            (score = (hidden @ W_score) . h_t
                                            collapses to hidden . (W_score @ h_t))
    score[t] = hidden[b,t,:] . u[b]        (DVE fp16 mul + pairwise-add tree)
    p[t]     = exp(score[t] - 50)          (ScalarE -> bf16, fused accum -> q)
    s        = sum_t p[t]                  (PE ones-matmul over q)
    ctx      = (sum_t p[t] * hidden[b,t,:]) / s
               (PE: bf16 p column as 1-col stationary vs fp16 y; the 1/s
                normalization folds into the PSUM->SBUF copy via ACT scale)
    out[b]   = tanh([ctx, h_t[b]] @ W_att)

bf16 p is overflow-safe (fp32-range exponent), so the context matmuls start
right after exp -- the softmax denominator (s -> 1/s) is computed concurrently
and only gates the tiny ctx_row copy.

Sharding: data-parallel over batch, 16 batches per core on 8 cores; weights
replicated.  hidden_states is read from HBM exactly once (fp32), cast to fp16
during the DMA (SWDGE cast), and never transposed.

Pipeline design:
  - The y16 load flood starts immediately; the small setup DMAs (ident, ht,
    wst, watt) ride the sync HWDGE ring concurrently (no dependent DMAs on
    that ring -> no HWDGE FIFO blocking, and nothing else queues on SDMA
    during the flood).
  - All u[b] / broadcast work happens on PE+ACT only (M=1 matmuls from h_t^T,
    then K=1 broadcast matmuls); DMAs inside the flood window would see
    ~20 us completion latency, so none are issued.
  - softmax sum via PE ones-matmul; reciprocal on DVE, emitted after the next
    batch's big mul so the DVE FIFO never head-of-line blocks.
  - last batch's load is split into 4 chunks so its score pipeline overlaps
    the tail of the flood.
"""

import sys

import numpy as np

_TRN_REPO = "/opt/trn_rl_repo"
if _TRN_REPO not in sys.path:
    sys.path.insert(0, _TRN_REPO)

import concourse.bass as bass
import concourse.bacc as bacc
import concourse.tile as tile
from concourse import mybir
from concourse.bass_utils import run_bass_kernel_spmd

N_CORES = 8
B, T, H = 128, 2048, 256
NB = B // N_CORES  # batches per core
P = 128  # SBUF partitions
TT = T // P  # t-tiles per batch
OUT_D = 128
EXP_SHIFT = -50.0  # keeps exp() in fp32/bf16 range; cancels in the softmax ratio

NCH = 4  # last batch is loaded/scored in NCH chunks to shorten the tail
CTT = TT // NCH

F32 = mybir.dt.float32
F16 = mybir.dt.float16
BF16 = mybir.dt.bfloat16


def _build_kernel(nc: bass.Bass, tc: "tile.TileContext", hidden, wst, watt, ident, out):
    add = mybir.AluOpType.add

    from contextlib import ExitStack

    with ExitStack() as ctx:
        const = ctx.enter_context(tc.tile_pool(name="const", bufs=1))
        ybufs = ctx.enter_context(tc.tile_pool(name="ybufs", bufs=8))
        sc = ctx.enter_context(tc.tile_pool(name="sc", bufs=3))
        psum_t = ctx.enter_context(tc.tile_pool(name="psum_t", bufs=2, space="PSUM"))
        psum_u = ctx.enter_context(tc.tile_pool(name="psum_u", bufs=2, space="PSUM"))
        psum_p = ctx.enter_context(tc.tile_pool(name="psum_p", bufs=1, space="PSUM"))

        # ---- constants (no DMA needed) -------------------------------------
        ones_row16 = const.tile([1, P], F16, tag="ones_row16")
        nc.vector.memset(ones_row16, 1.0)
        ones128 = const.tile([P, P], F32, tag="ones128")
        nc.vector.memset(ones128, 1.0)
        ones_col1 = const.tile([1, 1], F32, tag="ones_col1")
        nc.vector.memset(ones_col1, 1.0)
        shift_col = const.tile([P, 1], F32, tag="shift_col")
        nc.vector.memset(shift_col, EXP_SHIFT)

        # ---- setup DMAs: all independent, all on the sync HWDGE ring -------
        ident_sb = const.tile([16, 16], F32, tag="ident")
        nc.sync.dma_start(out=ident_sb, in_=ident[:, :])
        ht_sb = const.tile([NB, H], F32, tag="ht")
        nc.sync.dma_start(out=ht_sb, in_=hidden[:, T - 1, :])
        wst_sb = const.tile([P, 2, H], F32, tag="wst")  # W_score^T as [k, kk, h]
        nc.sync.dma_start(out=wst_sb, in_=wst.rearrange("(kk p) h -> p kk h", p=P))
        watt_sb = const.tile([P, 4, OUT_D], F32, tag="watt")  # W_att as [d, dd, j]
        nc.sync.dma_start(out=watt_sb, in_=watt.rearrange("(dd p) j -> p dd j", p=P))

        # ---- y16 load flood (SWDGE cast fp32->fp16), starts immediately ----
        ylist = {}
        for k in range(NB - 1):
            y = ybufs.tile([P, TT, H], F16, tag="y16", name=f"y16_{k}")
            nc.gpsimd.dma_start(
                out=y, in_=hidden[k].rearrange("(p i) h -> p i h", i=TT)
            )
            ylist[k] = y
        ychunks = []
        hlast = hidden[NB - 1].rearrange("(p i) h -> p i h", i=TT)
        for c in range(NCH):
            yc = ybufs.tile([P, CTT, H], F16, tag="y16c", name=f"y16c_{c}")
            nc.gpsimd.dma_start(out=yc, in_=hlast[:, c * CTT : (c + 1) * CTT, :])
            ychunks.append(yc)

        # ---- h_t^T, then per-batch u and its broadcast (PE+ACT only) -------
        htT_sb = const.tile([P, 2, NB], F32, tag="htT")  # h_t^T halves [k, half, b]
        for half in range(2):
            ps_tr = psum_t.tile([P, NB], F32, tag="ptmp", name=f"ps_tr{half}")
            nc.tensor.matmul(
                ps_tr,
                lhsT=ht_sb[:, half * P : (half + 1) * P],
                rhs=ident_sb,
                start=True,
                stop=True,
            )
            nc.scalar.copy(out=htT_sb[:, half, :], in_=ps_tr)

        # u[b] = h_t[b] @ W_score^T via M=1 matmuls (keeps everything at
        # partition 0); then broadcast to all 128 partitions via a K=1
        # matmul.  No DMAs -> nothing contends with the flood.
        ubc_all = const.tile([P, NB, H], F16, tag="ubc_all")
        for b in range(NB):
            ps_ub = psum_u.tile([1, H], F32, tag="pub", name=f"pub{b}")
            for half in range(2):
                nc.tensor.matmul(
                    ps_ub,
                    lhsT=htT_sb[:, half, b : b + 1],
                    rhs=wst_sb[:, half, :],
                    start=(half == 0),
                    stop=(half == 1),
                )
            u16b = sc.tile([1, H], F16, tag="u16b", name=f"u16b{b}")
            nc.scalar.copy(out=u16b, in_=ps_ub)
            ps_ubc = psum_t.tile([P, H], F32, tag="ptmp", name=f"pubc{b}")
            nc.tensor.matmul(ps_ubc, lhsT=ones_row16, rhs=u16b, start=True, stop=True)
            nc.scalar.copy(out=ubc_all[:, b, :], in_=ps_ubc)

        # ---- persistent PSUM accumulators for ctx^T ------------------------
        ctxT_ps = [
            psum_p.tile([P, NB], F32, tag=f"ctxT{j}", name=f"ctxT{j}")
            for j in range(2)
        ]

        # ---- per-batch score pipeline --------------------------------------
        # t = p*TT + i block mapping gives 16KB-contiguous DMA runs per
        # partition (softmax/context are t-permutation-invariant).
        state = {}  # batch -> dict of tiles needed by the deferred stages

        def ubc_rep(b, rep):
            ubc = ubc_all[:, b, :]
            return bass.AP(
                tensor=ubc.tensor,
                offset=ubc.offset,
                ap=[list(ubc.ap[0]), [0, rep], list(ubc.ap[1])],
            )

        def emit_score_tail(b, z, score, nt):
            z1 = sc.tile([P, nt, 128], F16, tag="z1" if nt == TT else "z1c")
            nc.vector.tensor_add(z1, z[:, :, 0:128], z[:, :, 128:256])
            z2 = sc.tile([P, nt, 64], F16, tag="z2" if nt == TT else "z2c")
            nc.vector.tensor_add(z2, z1[:, :, 0:64], z1[:, :, 64:128])
            nc.vector.tensor_reduce(
                out=score, in_=z2, axis=mybir.AxisListType.X, op=add
            )

        def emit_exp_s(b, score):
            # p in bf16: overflow-safe unnormalized weights, feeds PE directly
            p_t = sc.tile([P, TT], BF16, tag="p")
            q = sc.tile([P, 1], F32, tag="q")
            nc.scalar.activation(
                out=p_t,
                in_=score,
                func=mybir.ActivationFunctionType.Exp,
                bias=shift_col,
                scale=1.0,
                accum_out=q,
            )
            s_ps = psum_u.tile([P, 1], F32, tag="pub")
            nc.tensor.matmul(s_ps, lhsT=ones128, rhs=q, start=True, stop=True)
            state[b] = {"p": p_t, "s_ps": s_ps}

        def emit_recip(b):
            rs = sc.tile([P, 1], F32, tag="rs")
            nc.vector.reciprocal(out=rs, in_=state[b]["s_ps"])
            state[b]["rs"] = rs

        def emit_ctx(b):
            p_t = state[b]["p"]
            ctx_ps = psum_t.tile([1, H], F32, tag="ptmp", name=f"ctx{b}")
            if b < NB - 1:
                y16 = ylist.pop(b)
                for i in range(TT):
                    nc.tensor.matmul(
                        ctx_ps,
                        lhsT=p_t[:, i : i + 1],
                        rhs=y16[:, i, :],
                        start=(i == 0),
                        stop=(i == TT - 1),
                    )
            else:
                for i in range(TT):
                    nc.tensor.matmul(
                        ctx_ps,
                        lhsT=p_t[:, i : i + 1],
                        rhs=ychunks[i // CTT][:, i % CTT, :],
                        start=(i == 0),
                        stop=(i == TT - 1),
                    )
            state[b]["ctx_ps"] = ctx_ps

        def emit_ctx_row(b):
            # normalization by 1/s happens here, on the [1, 256] row only
            ctx_row = sc.tile([1, H], F32, tag="ctx_row")
            nc.scalar.mul(out=ctx_row, in_=state[b]["ctx_ps"], mul=state[b]["rs"][0:1, :])
            state[b]["ctx_row"] = ctx_row

        def emit_scatter(b):
            ctx_row = state[b]["ctx_row"]
            for j in range(2):
                nc.tensor.matmul(
                    ctxT_ps[j][:, b : b + 1],
                    lhsT=ctx_row[:, j * P : (j + 1) * P],
                    rhs=ones_col1,
                    start=True,
                    stop=True,
                )
            del state[b]

        for b in range(NB):
            if b < NB - 1:
                z = sc.tile([P, TT, H], F16, tag="z")
                nc.vector.tensor_mul(z, ylist[b], ubc_rep(b, TT))
                if b >= 1:
                    emit_recip(b - 1)
                    emit_ctx(b - 1)
                score = sc.tile([P, TT], F32, tag="score")
                emit_score_tail(b, z, score, TT)
            else:
                score = sc.tile([P, TT], F32, tag="score")
                for c in range(NCH):
                    zc = sc.tile([P, CTT, H], F16, tag="zc")
                    nc.vector.tensor_mul(zc, ychunks[c], ubc_rep(b, CTT))
                    if c == 0:
                        emit_recip(b - 1)
                        emit_ctx(b - 1)
                    emit_score_tail(b, zc, score[:, c * CTT : (c + 1) * CTT], CTT)
            emit_exp_s(b, score)
            if b >= 1:
                emit_ctx_row(b - 1)
                emit_scatter(b - 1)

        bl = NB - 1
        emit_recip(bl)
        emit_ctx(bl)
        emit_ctx_row(bl)
        emit_scatter(bl)

        # ---- finalize: concat with h_t, @W_att, tanh -----------------------
        preT = sc.tile([P, 2, NB], F32, tag="preT")
        for j in range(2):
            nc.scalar.copy(out=preT[:, j, :], in_=ctxT_ps[j])

        out_ps = psum_t.tile([NB, OUT_D], F32, tag="ptmp")
        for dd in range(4):
            lhsT = preT[:, dd, :] if dd < 2 else htT_sb[:, dd - 2, :]
            nc.tensor.matmul(
                out_ps,
                lhsT=lhsT,
                rhs=watt_sb[:, dd, :],
                start=(dd == 0),
                stop=(dd == 3),
            )
        out_sb = sc.tile([NB, OUT_D], F32, tag="out_sb")
        nc.scalar.activation(
            out=out_sb, in_=out_ps, func=mybir.ActivationFunctionType.Tanh
        )
        nc.sync.dma_start(out=out[:, :], in_=out_sb)


_NC_CACHE = {}


def _get_nc():
    if "nc" not in _NC_CACHE:
        nc = bacc.Bacc("TRN2", target_bir_lowering=False, debug=False)
        hidden = nc.declare_dram_parameter("hidden", [NB, T, H], F32, isOutput=False)
        wst = nc.declare_dram_parameter("w_score_t", [H, H], F32, isOutput=False)
        watt = nc.declare_dram_parameter("w_att", [2 * H, OUT_D], F32, isOutput=False)
        ident = nc.declare_dram_parameter("ident16", [16, 16], F32, isOutput=False)
        out = nc.declare_dram_parameter("out", [NB, OUT_D], F32, isOutput=True)
        with tile.TileContext(nc) as tc:
            _build_kernel(nc, tc, hidden, wst, watt, ident, out)
        nc.compile()
        _NC_CACHE["nc"] = nc
    return _NC_CACHE["nc"]


def _run(hidden_states, W_score, W_att, trace=False, trace_kwargs=None):
    hidden_states = np.ascontiguousarray(np.asarray(hidden_states, dtype=np.float32))
    W_score = np.asarray(W_score, dtype=np.float32)
    W_att = np.ascontiguousarray(np.asarray(W_att, dtype=np.float32))
    wst = np.ascontiguousarray(W_score.T)
    ident = np.eye(16, dtype=np.float32)

    nc = _get_nc()
    in_maps = []
    for c in range(N_CORES):
        in_maps.append(
            {
                "hidden": hidden_states[c * NB : (c + 1) * NB],
                "w_score_t": wst,
                "w_att": W_att,
                "ident16": ident,
            }
        )
    kwargs = {}
    if trace:
        kwargs["trace"] = True
        if trace_kwargs:
            kwargs.update(trace_kwargs)
    res = run_bass_kernel_spmd(nc, in_maps, list(range(N_CORES)), **kwargs)
    out = np.concatenate([res.results[c]["out"] for c in range(N_CORES)], axis=0)
    return out, res


def kernel(hidden_states, W_score, W_att):
    out, _ = _run(hidden_states, W_score, W_att, trace=False)
    return out
